# revision 21
# baseline (speedup 1.0000x reference)
"""ContourLoss on 8 Trainium2 NeuronCores (data parallel over batch B=8).

Device work per core (one sample), raw per-engine programs (no TileContext):
  - Intersection grid q12/q34 via small-K bf16 matmuls with a 3-product
    bf16 split (hh+mh+hm): K12=20, K34=18 (~1.5e-5 relative error on q).
    The two matmul streams live on DISJOINT PE row-quadrants (features at
    SBUF partitions 0:20 and 32:50 -> tile_position rows 0 / 32) and
    execute CONCURRENTLY, ~2x the serial column rate.
  - Per superblock (<=1024 cols, PSUM slot ring of 2): one scalar-engine
    sigmoid over BOTH q regions (strided [p,2,N] access), then DVE bf16
    product s1*s2 and a bf16 accumulate, software-pipelined so adds trail
    their product by 3 steps: no same-engine RAW pair is adjacent and no
    DVE pipe drains are needed.  Triangular mask (j>=i+2) multiplies the
    product's leading 132 columns (DVE, 0/1 bf16 tile).
  - The two bf16 accumulators ship raw via a second output DMA (the host
    sums them); the tail DMAs issue in parallel on the Sync and GpSimd
    queues with no completion wait (the framework postamble covers the
    in-flight latency).  Sub-512 tail chunks from different bands are
    bin-packed into shared PSUM slots (segments at packed column offsets
    share one sigmoid/product/accumulate chain).
  - CE avoids Exp/Ln (second ACT table would cost 2x 1.28us): the device
    ships raw sigma(-cp) values; the host finishes the logsumexp as
    ln(sum_c 1/sigma(-cp_c) - 4).  Every ACT op lives in the single
    sigmoid table, prefetched at t~0 by a dummy op.
  - Input DMAs split across Sync (HWDGE: features, gates the PE) and
    GpSimd (SWDGE: tri, tok); output DMA on the idle Sync engine.
  - Host does O(B*S) prep (compaction, split features, packing), the
    final scalar arithmetic, and the wrap-pair (0, n_seg-1) exclusion.
"""

import numpy as np

RETINA = 224.0
NUM_CLASSES = 4
B = 8
S = 2048
M = S - 1
NCORES = 8
BIG = 1.0e13
K34 = 18      # 6 features x 3 split products
K12 = K34 + 2 # + inv/one validity rows
NCOLS = 68
GS_COL0 = 4    # cols 4:68 = raw sigma(-cp) (64); host finishes CE
ISECT_COL0 = 19
TRIW = 132     # tri matmul width (covers all j-i<2 cells of a 128-band)

_CACHE = {}
TRACE_KWARGS = {}
LAST_RESULTS = None


# ---------------------------------------------------------------------------
# walrus in this environment accepts at most ONE sync-wait per instruction;
# split extras onto same-engine NoOps (semantically identical).
def _split_multi_waits(nc, max_waits=1):
    import concourse.mybir as mybir
    n_split = 0
    for fn in nc.m.functions:
        for blk in fn.blocks:
            out = []
            changed = False
            for inst in blk.instructions:
                si = inst.sync_info
                ow = list(si.on_wait) if (si is not None and si.on_wait) else []
                if len(ow) > max_waits:
                    for k, w in enumerate(ow[:-max_waits]):
                        out.append(mybir.InstNoOp(
                            name=f"{inst.name}_wsplit{k}",
                            engine=inst.engine,
                            ins=[], outs=[],
                            sync_info=mybir.SyncInfo(on_wait=[w],
                                                     on_update=[]),
                        ))
                        n_split += 1
                    si.on_wait = ow[-max_waits:]
                    changed = True
                out.append(inst)
            if changed:
                blk.instructions = out
    return n_split


def _sched(L, Jmax):
    """Superblocks, each a list of segments (i0, j0, n, coloff, leading).
    Full 512-wide chunks are singletons; sub-512 tail/leading chunks are
    bin-packed (<=512) so several bands share one PSUM slot and one
    sigmoid/product/accumulate chain."""
    fulls = []
    smalls = []
    for ib in range(L // 128):
        i0 = 128 * ib
        j0 = i0
        first = True
        while j0 < Jmax:
            N = min(512, Jmax - j0)
            (fulls if N == 512 else smalls).append((i0, j0, N, first))
            j0 += N
            first = False
    sbs = [[(i0, j0, n, 0, fr)] for (i0, j0, n, fr) in fulls]
    bins = []
    for (i0, j0, n, fr) in sorted(smalls, key=lambda x: -x[2]):
        for b in bins:
            used = sum(s[2] for s in b)
            if used + n <= 512:
                b.append((i0, j0, n, used, fr))
                break
        else:
            bins.append([(i0, j0, n, 0, fr)])
    bins.sort(key=lambda b: any(s[4] for s in b), reverse=True)
    return sbs + bins


def _split3(x):
    import ml_dtypes
    bf = ml_dtypes.bfloat16
    h = x.astype(bf).astype(np.float64)
    m = (x - h).astype(bf).astype(np.float64)
    return h, m


def _stack_A(X6):
    # products (hh, mh, hm): A rows [h, m, h]
    import ml_dtypes
    h, m = _split3(X6)
    return np.concatenate([h, m, h], 0).astype(ml_dtypes.bfloat16)


def _stack_B(X6):
    # products (hh, mh, hm): B rows [h, h, m]
    import ml_dtypes
    h, m = _split3(X6)
    return np.concatenate([h, h, m], 0).astype(ml_dtypes.bfloat16)


def _host_prep(pp, op, cp, ts, pm):
    tc_cls = ts[:, :, 4].astype(np.int32)
    tp = ts[:, :, :2]
    to = ts[:, :, 2:4]
    valid = ~pm
    nn = valid & (tc_cls != 0)

    per_core = []
    n_segs = []
    for b in range(B):
        order = np.argsort(~nn[b], kind="stable")
        pts = pp[b][order].astype(np.float64)
        n = int(nn[b].sum())
        n_seg = n - 1
        n_segs.append(n_seg)
        if n > 0:
            pts = pts - pts[:n].mean(axis=0)
        sx, sy = pts[:-1, 0], pts[:-1, 1]
        eX, eY = pts[1:, 0], pts[1:, 1]
        ex, ey = eX - sx, eY - sy
        c = ex * sy - ey * sx
        g0, g1, g2 = ex, -ey, -c
        one = np.ones(M)
        U6 = np.stack([g0 * g0, g1 * g1, g2 * g2,
                       g0 * g1, g0 * g2, g1 * g2], 0)
        V6 = np.stack([sy * eY, sx * eX, one,
                       sy * eX + sx * eY,
                       sy + eY,
                       sx + eX], 0)
        inv = (np.arange(M) >= max(n_seg, 0)).astype(np.float64) * BIG
        import ml_dtypes
        bfdt = ml_dtypes.bfloat16
        A12 = np.concatenate([_stack_A(V6), inv[None], one[None]],
                             0).astype(bfdt)
        B12 = np.concatenate([_stack_B(U6), one[None], inv[None]],
                             0).astype(bfdt)
        per_core.append(dict(n=n, n_seg=n_seg,
                             A12=A12, B12=B12,
                             A34=_stack_A(U6), B34=_stack_B(V6)))
    return tc_cls, tp, to, valid, nn, per_core, n_segs


def _build(L, sbs):
    import concourse.bass as bass
    from concourse import mybir

    f32 = mybir.dt.float32
    bf16 = mybir.dt.bfloat16
    ALU = mybir.AluOpType
    ACT = mybir.ActivationFunctionType
    AX = mybir.AxisListType

    NSB = len(sbs)
    assert GS_COL0 + 16 <= NCOLS

    nc = bass.Bass()
    # packed split features (bf16), PE-quadrant layout:
    #   partitions 0:20  = [A12 | B12]   (K12 rows)
    #   partitions 32:50 = [A34 | B34]   (K34 rows; PE tile row-offset 32 so
    #   the q12 and q34 matmuls run on disjoint PE row-quadrants)
    d_feat = nc.dram_tensor("feat", [64, 2 * L], bf16, kind="ExternalInput")
    # triangular keep-mask for the leading chunk of each band (j-i >= 2)
    d_tri = nc.dram_tensor("tri", [128, TRIW], bf16, kind="ExternalInput")
    # per-token data: pp*nn | tp*nn | op | to*nn | cp | onehot*valid
    d_tok = nc.dram_tensor("tok", [128, 256], f32, kind="ExternalInput")
    d_out = nc.dram_tensor("partials", [128, NCOLS], f32,
                           kind="ExternalOutput")
    d_acc = nc.dram_tensor("accs", [128, 2048], bf16,
                           kind="ExternalOutput")

    from contextlib import ExitStack
    with ExitStack() as ctx:
        feat = ctx.enter_context(nc.sbuf_tensor([64, 2 * L], bf16))
        tri = ctx.enter_context(nc.sbuf_tensor([128, TRIW], bf16))
        tok = ctx.enter_context(nc.sbuf_tensor([128, 256], f32))
        cols = ctx.enter_context(nc.sbuf_tensor([128, NCOLS], f32))
        s4 = ctx.enter_context(nc.sbuf_tensor([128, 64], f32))
        r4 = ctx.enter_context(nc.sbuf_tensor([128, 64], f32))
        dpt = ctx.enter_context(nc.sbuf_tensor([128, 32], f32))
        junko = ctx.enter_context(nc.sbuf_tensor([128, 32], f32))
        junks = ctx.enter_context(nc.sbuf_tensor([128, 64], f32))
        junk2 = ctx.enter_context(nc.sbuf_tensor([128, 64], f32))
        sg = ctx.enter_context(nc.sbuf_tensor([128, 4096], bf16))
        prod = ctx.enter_context(nc.sbuf_tensor([128, 4 * 1024], bf16))
        acc = ctx.enter_context(nc.sbuf_tensor([128, 2 * 1024], bf16))
        qps = ctx.enter_context(nc.psum_tensor([128, 4096], f32))
        dma_f0 = ctx.enter_context(nc.semaphore("dma_f0"))
        dma_f1 = ctx.enter_context(nc.semaphore("dma_f1"))
        dma_tri = ctx.enter_context(nc.semaphore("dma_tri"))
        dma_tok = ctx.enter_context(nc.semaphore("dma_tok"))
        dma_out = ctx.enter_context(nc.semaphore("dma_out"))
        pe_sem = ctx.enter_context(nc.semaphore("pe_sem"))
        sig_sem = ctx.enter_context(nc.semaphore("sig_sem"))
        pdone_sem = ctx.enter_context(nc.semaphore("pdone_sem"))
        ce_sem = ctx.enter_context(nc.semaphore("ce_sem"))
        colw_sem = ctx.enter_context(nc.semaphore("colw_sem"))
        acc_sem = ctx.enter_context(nc.semaphore("acc_sem"))
        block = ctx.enter_context(nc.Block(no_gpsimd_drain=True))

        fv_d = d_feat[:].rearrange("k (s l) -> k s l", s=2)
        fv_s = feat[:].rearrange("k (s l) -> k s l", s=2)
        ppn, tpn = tok[:, 0:32], tok[:, 32:64]
        opr, ton = tok[:, 64:96], tok[:, 96:128]
        cp4, ohv = tok[:, 128:192], tok[:, 192:256]

        # features on HWDGE (gates the tensor engine); tri+tok on SWDGE
        @block.sync
        def _(sp):
            sp.dma_start(out=fv_s[:, :, 0:512],
                         in_=fv_d[:, :, 0:512]).then_inc(dma_f0, 16)
            if L > 512:
                sp.dma_start(out=fv_s[:, :, 512:L],
                             in_=fv_d[:, :, 512:L]).then_inc(dma_f1, 16)
            sp.wait_ge(acc_sem, NSB)
            sp.dma_start(out=d_acc[:], in_=acc[:]).then_inc(dma_out, 16)

        @block.gpsimd
        def _(gp):
            gp.dma_start(out=tri[:], in_=d_tri[:]).then_inc(dma_tri, 16)
            gp.dma_start(out=tok[:], in_=d_tok[:]).then_inc(dma_tok, 16)
            gp.wait_ge(acc_sem, NSB)
            gp.wait_ge(colw_sem, 4)
            gp.dma_start(out=d_out[:], in_=cols[:]).then_inc(dma_out, 16)

        @block.tensor
        def _(pe):
            waited = {"f0": False, "f1": False}
            for c, segs in enumerate(sbs):
                need_f1 = any((j0 + n > 512) or (i0 + 128 > 512)
                              for (i0, j0, n, off, fr) in segs)
                if not waited["f0"]:
                    pe.wait_ge(dma_f0, 16)
                    waited["f0"] = True
                if need_f1 and not waited["f1"]:
                    pe.wait_ge(dma_f1, 16)
                    waited["f1"] = True
                if c >= 4:
                    pe.wait_ge(sig_sem, c - 3)
                base = 1024 * (c % 4)
                for (i0, j0, n, off, fr) in segs:
                    nc.tensor.matmul(
                        qps[:, base + off:base + off + n],
                        feat[0:K12, 0 * L + i0:0 * L + i0 + 128],
                        feat[0:K12, 1 * L + j0:1 * L + j0 + n],
                        start=True, stop=True).then_inc(pe_sem, 1)
                    nc.tensor.matmul(
                        qps[:, base + 512 + off:base + 512 + off + n],
                        feat[32:32 + K34, 0 * L + i0:0 * L + i0 + 128],
                        feat[32:32 + K34, 1 * L + j0:1 * L + j0 + n],
                        start=True, stop=True).then_inc(pe_sem, 1)

        @block.scalar
        def _(act):
            cum = []
            t = 0
            for segs in sbs:
                t += 2 * len(segs)
                cum.append(t)
            # dummy: pulls the sigmoid ACT table at t~0 (reads garbage)
            nc.scalar.activation(out=junk2[:, 0:1], in_=s4[:, 0:1],
                                 func=ACT.Sigmoid)
            for c, segs in enumerate(sbs):
                N = sum(s[2] for s in segs)
                if c >= 4:
                    act.wait_ge(pdone_sem, c - 3)
                act.wait_ge(pe_sem, cum[c])
                base = 1024 * (c % 4)
                qv = qps[:, base:base + 1024].rearrange(
                    "p (two n) -> p two n", two=2)[:, :, 0:N]
                sv = sg[:, base:base + 2 * N].rearrange(
                    "p (two n) -> p two n", two=2)
                nc.scalar.activation(out=sv, in_=qv, func=ACT.Sigmoid,
                                     scale=-0.01).then_inc(sig_sem, 1)
                if c == 0:
                    act.wait_ge(dma_tok, 16)
                    nc.scalar.activation(out=cols[:, GS_COL0:GS_COL0 + 64],
                                         in_=cp4, func=ACT.Sigmoid,
                                         scale=-1.0).then_inc(colw_sem, 1)

        @block.vector
        def _(dve):
            # software-pipelined: adds trail their mult by 3 sb-steps so no
            # same-engine RAW pair is ever adjacent (no pipe drains needed)
            dve.wait_ge(dma_tok, 16)
            nc.vector.tensor_tensor(out=dpt[:], in0=ppn, in1=tpn,
                                    op=ALU.subtract)
            nc.vector.tensor_tensor(out=junko[:], in0=opr, in1=ton,
                                    op=ALU.mult)
            nc.vector.tensor_tensor(out=junks[:], in0=cp4, in1=ohv,
                                    op=ALU.mult)
            nc.vector.tensor_tensor(out=junk2[:, 0:32], in0=dpt[:],
                                    in1=dpt[:], op=ALU.mult)
            nc.vector.tensor_reduce(out=cols[:, 1:2], in_=junko[:],
                                    axis=AX.X, op=ALU.add).then_inc(
                                        colw_sem, 1)
            nc.vector.tensor_reduce(out=cols[:, 2:3], in_=junks[:],
                                    axis=AX.X, op=ALU.add).then_inc(
                                        colw_sem, 1)
            nc.vector.tensor_reduce(out=cols[:, 0:1], in_=junk2[:, 0:32],
                                    axis=AX.X, op=ALU.add).then_inc(
                                        colw_sem, 1)

            def emit_add(k):
                kn = sum(s[2] for s in sbs[k])
                slot = 1024 * (k % 2)
                with nc.allow_low_precision(reason="bf16 grid accumulate"):
                    if k < 2:
                        if kn < 1024:
                            nc.vector.memset(acc[:, slot + kn:slot + 1024],
                                             0.0)
                        nc.vector.tensor_copy(
                            out=acc[:, slot:slot + kn],
                            in_=prod[:, 1024 * (k % 4):1024 * (k % 4) + kn],
                        ).then_inc(acc_sem, 1)
                    else:
                        nc.vector.tensor_tensor(
                            out=acc[:, slot:slot + kn],
                            in0=acc[:, slot:slot + kn],
                            in1=prod[:, 1024 * (k % 4):1024 * (k % 4) + kn],
                            op=ALU.add).then_inc(acc_sem, 1)

            def emit_tri(k):
                for (i0, j0, n, off, fr) in sbs[k]:
                    if not fr:
                        continue
                    W = min(TRIW, n)
                    po = 1024 * (k % 4) + off
                    nc.vector.tensor_tensor(
                        out=prod[:, po:po + W],
                        in0=prod[:, po:po + W],
                        in1=tri[:, 0:W], op=ALU.mult)

            def has_tri(k):
                return any(s[4] for s in sbs[k])

            tri_waited = False
            next_add = 0
            for c, segs in enumerate(sbs):
                N = sum(s[2] for s in segs)
                dve.wait_ge(sig_sem, c + 1)
                base = 1024 * (c % 4)
                nc.vector.tensor_tensor(
                    out=prod[:, 1024 * (c % 4):1024 * (c % 4) + N],
                    in0=sg[:, base:base + N],
                    in1=sg[:, base + N:base + 2 * N],
                    op=ALU.mult).then_inc(pdone_sem, 1)
                if c >= 1 and has_tri(c - 1):
                    if not tri_waited:
                        dve.wait_ge(dma_tri, 16)
                        tri_waited = True
                    emit_tri(c - 1)
                while next_add <= c - 1 and (
                        next_add <= c - 2 or not has_tri(next_add)):
                    emit_add(next_add)
                    next_add += 1
            if has_tri(NSB - 1):
                emit_tri(NSB - 1)
                nc.vector.drain()
            while next_add < NSB:
                emit_add(next_add)
                next_add += 1

    _split_multi_waits(nc)
    return nc


def kernel(point_pred, orient_pred, class_pred, target_seq, padding_mask):
    pp = np.ascontiguousarray(np.asarray(point_pred, dtype=np.float32))
    op = np.ascontiguousarray(np.asarray(orient_pred, dtype=np.float32))
    cp = np.ascontiguousarray(np.asarray(class_pred, dtype=np.float32))
    ts = np.ascontiguousarray(np.asarray(target_seq, dtype=np.float32))
    pm = np.ascontiguousarray(np.asarray(padding_mask)).astype(bool)

    tc_cls, tp, to, valid, nn, per_core, n_segs = _host_prep(pp, op, cp, ts, pm)

    nsmax = max(max(n_segs), 1)
    L = max(128, -(-nsmax // 128) * 128)
    L = min(L, -(-M // 128) * 128)
    Jmax = min(-(-nsmax // 4) * 4, L)

    sbs = _sched(L, Jmax)
    key = (L, Jmax)
    if key not in _CACHE:
        _CACHE[key] = _build(L, sbs)
    nc = _CACHE[key]

    import ml_dtypes
    bfdt = ml_dtypes.bfloat16

    # triangular keep-mask for the leading chunk of each band
    tri_pack = (np.arange(TRIW)[None, :]
                >= np.arange(128)[:, None] + 2).astype(bfdt)

    eye = np.eye(NUM_CLASSES, dtype=np.float32)
    w = min(M, L)
    in_maps = []
    for b in range(B):
        pc = per_core[b]
        featpk = np.zeros((64, 2 * L), bfdt)
        featpk[0:K12, 0 * L:0 * L + w] = pc["A12"][:, :w]
        featpk[0:K12, 1 * L:1 * L + w] = pc["B12"][:, :w]
        featpk[32:32 + K34, 0 * L:0 * L + w] = pc["A34"][:, :w]
        featpk[32:32 + K34, 1 * L:1 * L + w] = pc["B34"][:, :w]
        if L > M:
            big_bf = bfdt(BIG)
            featpk[K34, 0 * L + M:1 * L] = big_bf
            featpk[K34 + 1, 0 * L + M:1 * L] = bfdt(1.0)
            featpk[K34, 1 * L + M:2 * L] = bfdt(1.0)
            featpk[K34 + 1, 1 * L + M:2 * L] = big_bf

        nf = nn[b].astype(np.float32)[:, None]
        vfb = valid[b].astype(np.float32)
        tokpk = np.concatenate([
            (pp[b] * nf).reshape(128, 32),
            (tp[b] * nf).reshape(128, 32),
            op[b].reshape(128, 32),
            (to[b] * nf).reshape(128, 32),
            cp[b].reshape(128, 64),
            (eye[tc_cls[b]] * vfb[:, None]).reshape(128, 64),
        ], axis=1).astype(np.float32)
        in_maps.append({
            "feat": np.ascontiguousarray(featpk),
            "tri": tri_pack,
            "tok": np.ascontiguousarray(tokpk),
        })

    from concourse.bass_utils import run_bass_kernel_spmd
    global LAST_RESULTS
    kw = dict(TRACE_KWARGS) if TRACE_KWARGS else {}
    res = run_bass_kernel_spmd(nc, in_maps, core_ids=list(range(NCORES)), **kw)
    LAST_RESULTS = res
    parts = [r["partials"] for r in res.results]

    f32 = np.float32
    NSB = len(sbs)
    pt_raw = f32(0); cos_sum = f32(0); sel_sum = f32(0)
    lse_sum = np.float64(0.0)
    isect_sum = np.float64(0.0)
    for b in range(B):
        p = parts[b].astype(np.float32)
        pt_raw += p[:, 0].sum(dtype=np.float32)
        cos_sum += p[:, 1].sum(dtype=np.float32)
        sel_sum += p[:, 2].sum(dtype=np.float32)
        s4h = p[:, GS_COL0:GS_COL0 + 64].astype(np.float64)
        gs = (1.0 / s4h).reshape(2048, 4).sum(axis=1)
        lse = np.log(np.maximum(gs - 4.0, 1e-300))
        lse_sum += (lse * valid[b]).sum()
        isect_sum += res.results[b]["accs"].astype(np.float64).sum()

    # host corrections on the device's own approximation (bf16-split min-s)
    def s_dev(pc, i, j):
        q12 = np.float32(np.dot(pc["A12"][:, i].astype(np.float32),
                                pc["B12"][:, j].astype(np.float32)))
        q34 = np.float32(np.dot(pc["A34"][:, i].astype(np.float32),
                                pc["B34"][:, j].astype(np.float32)))
        with np.errstate(over="ignore"):
            s1 = 1.0 / (1.0 + np.exp(np.clip(0.01 * np.float64(q12),
                                             -700.0, 700.0)))
            s2 = 1.0 / (1.0 + np.exp(np.clip(0.01 * np.float64(q34),
                                             -700.0, 700.0)))
        return s1 * s2

    cnt_total = 0
    corr = np.float64(0.0)
    for b in range(B):
        pc = per_core[b]
        n, n_seg = pc["n"], pc["n_seg"]
        if n < 4:
            continue
        cnt_total += (n_seg - 1) * (n_seg - 2) // 2 - 1
        # reference excludes the wrap pair (0, n_seg-1); device computed it
        corr -= s_dev(pc, 0, n_seg - 1)

    valid_cnt = f32(valid.sum())
    nn_cnt = f32(nn.sum())
    vden = max(valid_cnt, f32(1.0))
    nden = max(nn_cnt, f32(1.0))

    pt_loss = f32(pt_raw * f32(0.25 / (RETINA * RETINA)) / nden)
    orient_loss = f32((nn_cnt - cos_sum) / nden)
    cls_loss = f32((lse_sum - np.float64(sel_sum)) / vden)
    if cnt_total > 0:
        isect_loss = f32((isect_sum + corr) / cnt_total)
    else:
        isect_loss = f32(0.0)
    total = f32(pt_loss + f32(0.5) * orient_loss + cls_loss
                + f32(0.1) * isect_loss)
    return (total, pt_loss, orient_loss, cls_loss, isect_loss)


# revision 22
# speedup vs baseline: 1.0086x; 1.0086x over previous
"""ContourLoss on 8 Trainium2 NeuronCores (data parallel over batch B=8).

Device work per core (one sample), raw per-engine programs (no TileContext):
  - Intersection grid q12/q34 via small-K bf16 matmuls with a 3-product
    bf16 split (hh+mh+hm): K12=20, K34=18 (~1.5e-5 relative error on q).
    The two matmul streams live on DISJOINT PE row-quadrants (features at
    SBUF partitions 0:20 and 32:50 -> tile_position rows 0 / 32) and
    execute CONCURRENTLY, ~2x the serial column rate.
  - Per superblock (<=1024 cols, PSUM slot ring of 2): one scalar-engine
    sigmoid over BOTH q regions (strided [p,2,N] access), then DVE bf16
    product s1*s2 and a bf16 accumulate, software-pipelined so adds trail
    their product by 3 steps: no same-engine RAW pair is adjacent and no
    DVE pipe drains are needed.  Triangular mask (j>=i+2) multiplies the
    product's leading 132 columns (DVE, 0/1 bf16 tile).
  - The two bf16 accumulators ship raw via a second output DMA (the host
    sums them); the tail DMAs issue in parallel on the Sync and GpSimd
    queues with no completion wait (the framework postamble covers the
    in-flight latency).  Sub-512 tail chunks from different bands are
    bin-packed into shared PSUM slots (segments at packed column offsets
    share one sigmoid/product/accumulate chain).
  - CE avoids Exp/Ln (second ACT table would cost 2x 1.28us): the device
    ships raw sigma(-cp) values; the host finishes the logsumexp as
    ln(sum_c 1/sigma(-cp_c) - 4).  Every ACT op lives in the single
    sigmoid table, prefetched at t~0 by a dummy op.
  - Input DMAs split across Sync (HWDGE: features, gates the PE) and
    GpSimd (SWDGE: tri, tok); output DMA on the idle Sync engine.
  - Host does O(B*S) prep (compaction, split features, packing), the
    final scalar arithmetic, and the wrap-pair (0, n_seg-1) exclusion.
"""

import numpy as np

RETINA = 224.0
NUM_CLASSES = 4
B = 8
S = 2048
M = S - 1
NCORES = 8
BIG = 1.0e13
K34 = 18      # 6 features x 3 split products
K12 = K34 + 2 # + inv/one validity rows
NCOLS = 68
GS_COL0 = 4    # cols 4:68 = raw sigma(-cp) (64); host finishes CE
ISECT_COL0 = 19
TRIW = 132     # tri matmul width (covers all j-i<2 cells of a 128-band)

_CACHE = {}
TRACE_KWARGS = {}
LAST_RESULTS = None


# ---------------------------------------------------------------------------
# walrus in this environment accepts at most ONE sync-wait per instruction;
# split extras onto same-engine NoOps (semantically identical).
def _split_multi_waits(nc, max_waits=1):
    import concourse.mybir as mybir
    n_split = 0
    for fn in nc.m.functions:
        for blk in fn.blocks:
            out = []
            changed = False
            for inst in blk.instructions:
                si = inst.sync_info
                ow = list(si.on_wait) if (si is not None and si.on_wait) else []
                if len(ow) > max_waits:
                    for k, w in enumerate(ow[:-max_waits]):
                        out.append(mybir.InstNoOp(
                            name=f"{inst.name}_wsplit{k}",
                            engine=inst.engine,
                            ins=[], outs=[],
                            sync_info=mybir.SyncInfo(on_wait=[w],
                                                     on_update=[]),
                        ))
                        n_split += 1
                    si.on_wait = ow[-max_waits:]
                    changed = True
                out.append(inst)
            if changed:
                blk.instructions = out
    return n_split


def _sched(L, Jmax):
    """Superblocks, each a list of segments (i0, j0, n, coloff, leading).
    Full 512-wide chunks are singletons; sub-512 tail/leading chunks are
    bin-packed (<=512) so several bands share one PSUM slot and one
    sigmoid/product/accumulate chain."""
    fulls = []
    smalls = []
    for ib in range(L // 128):
        i0 = 128 * ib
        j0 = i0
        first = True
        while j0 < Jmax:
            N = min(512, Jmax - j0)
            (fulls if N == 512 else smalls).append((i0, j0, N, first))
            j0 += N
            first = False
    sbs = [[(i0, j0, n, 0, fr)] for (i0, j0, n, fr) in fulls]
    bins = []
    for (i0, j0, n, fr) in sorted(smalls, key=lambda x: -x[2]):
        for b in bins:
            used = sum(s[2] for s in b)
            if used + n <= 512:
                b.append((i0, j0, n, used, fr))
                break
        else:
            bins.append([(i0, j0, n, 0, fr)])
    return sbs + bins


def _split3(x):
    import ml_dtypes
    bf = ml_dtypes.bfloat16
    h = x.astype(bf).astype(np.float64)
    m = (x - h).astype(bf).astype(np.float64)
    return h, m


def _stack_A(X6):
    # products (hh, mh, hm): A rows [h, m, h]
    import ml_dtypes
    h, m = _split3(X6)
    return np.concatenate([h, m, h], 0).astype(ml_dtypes.bfloat16)


def _stack_B(X6):
    # products (hh, mh, hm): B rows [h, h, m]
    import ml_dtypes
    h, m = _split3(X6)
    return np.concatenate([h, h, m], 0).astype(ml_dtypes.bfloat16)


def _host_prep(pp, op, cp, ts, pm):
    tc_cls = ts[:, :, 4].astype(np.int32)
    tp = ts[:, :, :2]
    to = ts[:, :, 2:4]
    valid = ~pm
    nn = valid & (tc_cls != 0)

    per_core = []
    n_segs = []
    for b in range(B):
        order = np.argsort(~nn[b], kind="stable")
        pts = pp[b][order].astype(np.float64)
        n = int(nn[b].sum())
        n_seg = n - 1
        n_segs.append(n_seg)
        if n > 0:
            pts = pts - pts[:n].mean(axis=0)
        sx, sy = pts[:-1, 0], pts[:-1, 1]
        eX, eY = pts[1:, 0], pts[1:, 1]
        ex, ey = eX - sx, eY - sy
        c = ex * sy - ey * sx
        g0, g1, g2 = ex, -ey, -c
        one = np.ones(M)
        U6 = np.stack([g0 * g0, g1 * g1, g2 * g2,
                       g0 * g1, g0 * g2, g1 * g2], 0)
        V6 = np.stack([sy * eY, sx * eX, one,
                       sy * eX + sx * eY,
                       sy + eY,
                       sx + eX], 0)
        inv = (np.arange(M) >= max(n_seg, 0)).astype(np.float64) * BIG
        import ml_dtypes
        bfdt = ml_dtypes.bfloat16
        A12 = np.concatenate([_stack_A(V6), inv[None], one[None]],
                             0).astype(bfdt)
        B12 = np.concatenate([_stack_B(U6), one[None], inv[None]],
                             0).astype(bfdt)
        per_core.append(dict(n=n, n_seg=n_seg,
                             A12=A12, B12=B12,
                             A34=_stack_A(U6), B34=_stack_B(V6)))
    return tc_cls, tp, to, valid, nn, per_core, n_segs


def _build(L, sbs):
    import concourse.bass as bass
    from concourse import mybir

    f32 = mybir.dt.float32
    bf16 = mybir.dt.bfloat16
    ALU = mybir.AluOpType
    ACT = mybir.ActivationFunctionType
    AX = mybir.AxisListType

    NSB = len(sbs)
    assert GS_COL0 + 16 <= NCOLS

    nc = bass.Bass()
    # packed split features (bf16), PE-quadrant layout:
    #   partitions 0:20  = [A12 | B12]   (K12 rows)
    #   partitions 32:50 = [A34 | B34]   (K34 rows; PE tile row-offset 32 so
    #   the q12 and q34 matmuls run on disjoint PE row-quadrants)
    d_feat = nc.dram_tensor("feat", [64, 2 * L], bf16, kind="ExternalInput")
    # triangular keep-mask for the leading chunk of each band (j-i >= 2)
    d_tri = nc.dram_tensor("tri", [128, TRIW], bf16, kind="ExternalInput")
    # per-token data: pp*nn | tp*nn | op | to*nn | cp | onehot*valid
    d_tok = nc.dram_tensor("tok", [128, 256], f32, kind="ExternalInput")
    d_out = nc.dram_tensor("partials", [128, NCOLS], f32,
                           kind="ExternalOutput")
    d_acc = nc.dram_tensor("accs", [128, 2048], bf16,
                           kind="ExternalOutput")

    from contextlib import ExitStack
    with ExitStack() as ctx:
        feat = ctx.enter_context(nc.sbuf_tensor([64, 2 * L], bf16))
        tri = ctx.enter_context(nc.sbuf_tensor([128, TRIW], bf16))
        tok = ctx.enter_context(nc.sbuf_tensor([128, 256], f32))
        cols = ctx.enter_context(nc.sbuf_tensor([128, NCOLS], f32))
        s4 = ctx.enter_context(nc.sbuf_tensor([128, 64], f32))
        r4 = ctx.enter_context(nc.sbuf_tensor([128, 64], f32))
        dpt = ctx.enter_context(nc.sbuf_tensor([128, 32], f32))
        junko = ctx.enter_context(nc.sbuf_tensor([128, 32], f32))
        junks = ctx.enter_context(nc.sbuf_tensor([128, 64], f32))
        junk2 = ctx.enter_context(nc.sbuf_tensor([128, 64], f32))
        sg = ctx.enter_context(nc.sbuf_tensor([128, 4096], bf16))
        prod = ctx.enter_context(nc.sbuf_tensor([128, 4 * 1024], bf16))
        acc = ctx.enter_context(nc.sbuf_tensor([128, 2 * 1024], bf16))
        qps = ctx.enter_context(nc.psum_tensor([128, 4096], f32))
        dma_f0 = ctx.enter_context(nc.semaphore("dma_f0"))
        dma_f1 = ctx.enter_context(nc.semaphore("dma_f1"))
        dma_tri = ctx.enter_context(nc.semaphore("dma_tri"))
        dma_tok = ctx.enter_context(nc.semaphore("dma_tok"))
        dma_out = ctx.enter_context(nc.semaphore("dma_out"))
        pe_sem = ctx.enter_context(nc.semaphore("pe_sem"))
        sig_sem = ctx.enter_context(nc.semaphore("sig_sem"))
        pdone_sem = ctx.enter_context(nc.semaphore("pdone_sem"))
        ce_sem = ctx.enter_context(nc.semaphore("ce_sem"))
        colw_sem = ctx.enter_context(nc.semaphore("colw_sem"))
        acc_sem = ctx.enter_context(nc.semaphore("acc_sem"))
        block = ctx.enter_context(nc.Block(no_gpsimd_drain=True))

        fv_d = d_feat[:].rearrange("k (s l) -> k s l", s=2)
        fv_s = feat[:].rearrange("k (s l) -> k s l", s=2)
        ppn, tpn = tok[:, 0:32], tok[:, 32:64]
        opr, ton = tok[:, 64:96], tok[:, 96:128]
        cp4, ohv = tok[:, 128:192], tok[:, 192:256]

        # features on HWDGE (gates the tensor engine); tri+tok on SWDGE
        @block.sync
        def _(sp):
            sp.dma_start(out=fv_s[:, :, 0:512],
                         in_=fv_d[:, :, 0:512]).then_inc(dma_f0, 16)
            if L > 512:
                sp.dma_start(out=fv_s[:, :, 512:L],
                             in_=fv_d[:, :, 512:L]).then_inc(dma_f1, 16)
            sp.wait_ge(acc_sem, NSB)
            sp.dma_start(out=d_acc[:], in_=acc[:]).then_inc(dma_out, 16)

        @block.gpsimd
        def _(gp):
            gp.dma_start(out=tri[:], in_=d_tri[:]).then_inc(dma_tri, 16)
            gp.dma_start(out=tok[:], in_=d_tok[:]).then_inc(dma_tok, 16)
            gp.wait_ge(acc_sem, NSB)
            gp.wait_ge(colw_sem, 4)
            gp.dma_start(out=d_out[:], in_=cols[:]).then_inc(dma_out, 16)

        @block.tensor
        def _(pe):
            waited = {"f0": False, "f1": False}
            for c, segs in enumerate(sbs):
                need_f1 = any((j0 + n > 512) or (i0 + 128 > 512)
                              for (i0, j0, n, off, fr) in segs)
                if not waited["f0"]:
                    pe.wait_ge(dma_f0, 16)
                    waited["f0"] = True
                if need_f1 and not waited["f1"]:
                    pe.wait_ge(dma_f1, 16)
                    waited["f1"] = True
                if c >= 4:
                    pe.wait_ge(sig_sem, c - 3)
                base = 1024 * (c % 4)
                for (i0, j0, n, off, fr) in segs:
                    nc.tensor.matmul(
                        qps[:, base + off:base + off + n],
                        feat[0:K12, 0 * L + i0:0 * L + i0 + 128],
                        feat[0:K12, 1 * L + j0:1 * L + j0 + n],
                        start=True, stop=True).then_inc(pe_sem, 1)
                    nc.tensor.matmul(
                        qps[:, base + 512 + off:base + 512 + off + n],
                        feat[32:32 + K34, 0 * L + i0:0 * L + i0 + 128],
                        feat[32:32 + K34, 1 * L + j0:1 * L + j0 + n],
                        start=True, stop=True).then_inc(pe_sem, 1)

        @block.scalar
        def _(act):
            cum = []
            t = 0
            for segs in sbs:
                t += 2 * len(segs)
                cum.append(t)
            # dummy: pulls the sigmoid ACT table at t~0 (reads garbage)
            nc.scalar.activation(out=junk2[:, 0:1], in_=s4[:, 0:1],
                                 func=ACT.Sigmoid)
            for c, segs in enumerate(sbs):
                N = sum(s[2] for s in segs)
                if c >= 4:
                    act.wait_ge(pdone_sem, c - 3)
                act.wait_ge(pe_sem, cum[c])
                base = 1024 * (c % 4)
                qv = qps[:, base:base + 1024].rearrange(
                    "p (two n) -> p two n", two=2)[:, :, 0:N]
                sv = sg[:, base:base + 2 * N].rearrange(
                    "p (two n) -> p two n", two=2)
                nc.scalar.activation(out=sv, in_=qv, func=ACT.Sigmoid,
                                     scale=-0.01).then_inc(sig_sem, 1)
                if c == 0:
                    act.wait_ge(dma_tok, 16)
                    nc.scalar.activation(out=cols[:, GS_COL0:GS_COL0 + 64],
                                         in_=cp4, func=ACT.Sigmoid,
                                         scale=-1.0).then_inc(colw_sem, 1)

        @block.vector
        def _(dve):
            # software-pipelined: adds trail their mult by 3 sb-steps so no
            # same-engine RAW pair is ever adjacent (no pipe drains needed)
            dve.wait_ge(dma_tok, 16)
            nc.vector.tensor_tensor(out=dpt[:], in0=ppn, in1=tpn,
                                    op=ALU.subtract)
            nc.vector.tensor_tensor(out=junko[:], in0=opr, in1=ton,
                                    op=ALU.mult)
            nc.vector.tensor_tensor(out=junks[:], in0=cp4, in1=ohv,
                                    op=ALU.mult)
            nc.vector.tensor_tensor(out=junk2[:, 0:32], in0=dpt[:],
                                    in1=dpt[:], op=ALU.mult)
            nc.vector.tensor_reduce(out=cols[:, 1:2], in_=junko[:],
                                    axis=AX.X, op=ALU.add).then_inc(
                                        colw_sem, 1)
            nc.vector.tensor_reduce(out=cols[:, 2:3], in_=junks[:],
                                    axis=AX.X, op=ALU.add).then_inc(
                                        colw_sem, 1)
            nc.vector.tensor_reduce(out=cols[:, 0:1], in_=junk2[:, 0:32],
                                    axis=AX.X, op=ALU.add).then_inc(
                                        colw_sem, 1)

            def emit_add(k):
                kn = sum(s[2] for s in sbs[k])
                slot = 1024 * (k % 2)
                with nc.allow_low_precision(reason="bf16 grid accumulate"):
                    if k < 2:
                        if kn < 1024:
                            nc.vector.memset(acc[:, slot + kn:slot + 1024],
                                             0.0)
                        nc.vector.tensor_copy(
                            out=acc[:, slot:slot + kn],
                            in_=prod[:, 1024 * (k % 4):1024 * (k % 4) + kn],
                        ).then_inc(acc_sem, 1)
                    else:
                        nc.vector.tensor_tensor(
                            out=acc[:, slot:slot + kn],
                            in0=acc[:, slot:slot + kn],
                            in1=prod[:, 1024 * (k % 4):1024 * (k % 4) + kn],
                            op=ALU.add).then_inc(acc_sem, 1)

            def emit_tri(k):
                for (i0, j0, n, off, fr) in sbs[k]:
                    if not fr:
                        continue
                    W = min(TRIW, n)
                    po = 1024 * (k % 4) + off
                    nc.vector.tensor_tensor(
                        out=prod[:, po:po + W],
                        in0=prod[:, po:po + W],
                        in1=tri[:, 0:W], op=ALU.mult)

            def has_tri(k):
                return any(s[4] for s in sbs[k])

            tri_waited = False
            for c, segs in enumerate(sbs):
                N = sum(s[2] for s in segs)
                dve.wait_ge(sig_sem, c + 1)
                base = 1024 * (c % 4)
                nc.vector.tensor_tensor(
                    out=prod[:, 1024 * (c % 4):1024 * (c % 4) + N],
                    in0=sg[:, base:base + N],
                    in1=sg[:, base + N:base + 2 * N],
                    op=ALU.mult).then_inc(pdone_sem, 1)
                if c >= 2:
                    emit_add(c - 2)
                if c >= 1 and has_tri(c - 1):
                    if not tri_waited:
                        dve.wait_ge(dma_tri, 16)
                        tri_waited = True
                    emit_tri(c - 1)
            if has_tri(NSB - 1):
                emit_tri(NSB - 1)
            for k in (NSB - 2, NSB - 1):
                if k >= 0:
                    emit_add(k)

    _split_multi_waits(nc)
    return nc


def kernel(point_pred, orient_pred, class_pred, target_seq, padding_mask):
    pp = np.ascontiguousarray(np.asarray(point_pred, dtype=np.float32))
    op = np.ascontiguousarray(np.asarray(orient_pred, dtype=np.float32))
    cp = np.ascontiguousarray(np.asarray(class_pred, dtype=np.float32))
    ts = np.ascontiguousarray(np.asarray(target_seq, dtype=np.float32))
    pm = np.ascontiguousarray(np.asarray(padding_mask)).astype(bool)

    tc_cls, tp, to, valid, nn, per_core, n_segs = _host_prep(pp, op, cp, ts, pm)

    nsmax = max(max(n_segs), 1)
    L = max(128, -(-nsmax // 128) * 128)
    L = min(L, -(-M // 128) * 128)
    Jmax = min(-(-nsmax // 4) * 4, L)

    sbs = _sched(L, Jmax)
    key = (L, Jmax)
    if key not in _CACHE:
        _CACHE[key] = _build(L, sbs)
    nc = _CACHE[key]

    import ml_dtypes
    bfdt = ml_dtypes.bfloat16

    # triangular keep-mask for the leading chunk of each band
    tri_pack = (np.arange(TRIW)[None, :]
                >= np.arange(128)[:, None] + 2).astype(bfdt)

    eye = np.eye(NUM_CLASSES, dtype=np.float32)
    w = min(M, L)
    in_maps = []
    for b in range(B):
        pc = per_core[b]
        featpk = np.zeros((64, 2 * L), bfdt)
        featpk[0:K12, 0 * L:0 * L + w] = pc["A12"][:, :w]
        featpk[0:K12, 1 * L:1 * L + w] = pc["B12"][:, :w]
        featpk[32:32 + K34, 0 * L:0 * L + w] = pc["A34"][:, :w]
        featpk[32:32 + K34, 1 * L:1 * L + w] = pc["B34"][:, :w]
        if L > M:
            big_bf = bfdt(BIG)
            featpk[K34, 0 * L + M:1 * L] = big_bf
            featpk[K34 + 1, 0 * L + M:1 * L] = bfdt(1.0)
            featpk[K34, 1 * L + M:2 * L] = bfdt(1.0)
            featpk[K34 + 1, 1 * L + M:2 * L] = big_bf

        nf = nn[b].astype(np.float32)[:, None]
        vfb = valid[b].astype(np.float32)
        tokpk = np.concatenate([
            (pp[b] * nf).reshape(128, 32),
            (tp[b] * nf).reshape(128, 32),
            op[b].reshape(128, 32),
            (to[b] * nf).reshape(128, 32),
            cp[b].reshape(128, 64),
            (eye[tc_cls[b]] * vfb[:, None]).reshape(128, 64),
        ], axis=1).astype(np.float32)
        in_maps.append({
            "feat": np.ascontiguousarray(featpk),
            "tri": tri_pack,
            "tok": np.ascontiguousarray(tokpk),
        })

    from concourse.bass_utils import run_bass_kernel_spmd
    global LAST_RESULTS
    kw = dict(TRACE_KWARGS) if TRACE_KWARGS else {}
    res = run_bass_kernel_spmd(nc, in_maps, core_ids=list(range(NCORES)), **kw)
    LAST_RESULTS = res
    parts = [r["partials"] for r in res.results]

    f32 = np.float32
    NSB = len(sbs)
    pt_raw = f32(0); cos_sum = f32(0); sel_sum = f32(0)
    lse_sum = np.float64(0.0)
    isect_sum = np.float64(0.0)
    for b in range(B):
        p = parts[b].astype(np.float32)
        pt_raw += p[:, 0].sum(dtype=np.float32)
        cos_sum += p[:, 1].sum(dtype=np.float32)
        sel_sum += p[:, 2].sum(dtype=np.float32)
        s4h = p[:, GS_COL0:GS_COL0 + 64].astype(np.float64)
        gs = (1.0 / s4h).reshape(2048, 4).sum(axis=1)
        lse = np.log(np.maximum(gs - 4.0, 1e-300))
        lse_sum += (lse * valid[b]).sum()
        isect_sum += res.results[b]["accs"].astype(np.float64).sum()

    # host corrections on the device's own approximation (bf16-split min-s)
    def s_dev(pc, i, j):
        q12 = np.float32(np.dot(pc["A12"][:, i].astype(np.float32),
                                pc["B12"][:, j].astype(np.float32)))
        q34 = np.float32(np.dot(pc["A34"][:, i].astype(np.float32),
                                pc["B34"][:, j].astype(np.float32)))
        with np.errstate(over="ignore"):
            s1 = 1.0 / (1.0 + np.exp(np.clip(0.01 * np.float64(q12),
                                             -700.0, 700.0)))
            s2 = 1.0 / (1.0 + np.exp(np.clip(0.01 * np.float64(q34),
                                             -700.0, 700.0)))
        return s1 * s2

    cnt_total = 0
    corr = np.float64(0.0)
    for b in range(B):
        pc = per_core[b]
        n, n_seg = pc["n"], pc["n_seg"]
        if n < 4:
            continue
        cnt_total += (n_seg - 1) * (n_seg - 2) // 2 - 1
        # reference excludes the wrap pair (0, n_seg-1); device computed it
        corr -= s_dev(pc, 0, n_seg - 1)

    valid_cnt = f32(valid.sum())
    nn_cnt = f32(nn.sum())
    vden = max(valid_cnt, f32(1.0))
    nden = max(nn_cnt, f32(1.0))

    pt_loss = f32(pt_raw * f32(0.25 / (RETINA * RETINA)) / nden)
    orient_loss = f32((nn_cnt - cos_sum) / nden)
    cls_loss = f32((lse_sum - np.float64(sel_sum)) / vden)
    if cnt_total > 0:
        isect_loss = f32((isect_sum + corr) / cnt_total)
    else:
        isect_loss = f32(0.0)
    total = f32(pt_loss + f32(0.5) * orient_loss + cls_loss
                + f32(0.1) * isect_loss)
    return (total, pt_loss, orient_loss, cls_loss, isect_loss)
